# revision 26
# baseline (speedup 1.0000x reference)
"""Trainium2 Bass kernel for nn_DSVM (deep state-space variational model).

Strategy (self-contained; shapes hardcoded):
- 8-way data-parallel over batch B=128 -> 16 samples/core, SPMD, no
  collectives. Scalar losses come back as per-core partials summed on host.
- Transposed compute layout everywhere: [features, batch] with features on
  SBUF partitions. Weights are host-pre-transposed into lhsT [K_in, M_out]
  bf16 tiles and stay SBUF-resident for all 512 steps; activations stream as
  the moving operand.
- Sequential phases: (1) backward 2-layer GRU over reversed y, storing the
  top-layer output ob_t; (2) batched matmul P_ob = zp_W1[:, :H] @ ob_t for
  all t; (3) forward scan: posterior MLP (pipelined one step ahead),
  2-layer GRU, emission net, with gi+gh fused via PSUM accumulation and all
  biases folded into ones-row matmul tricks / precomputed fp32 bias tiles;
  (4) batched epilogue: prior MLP from stored z_prev, KLD/NLL reductions.
- Single ACT table set (natural_log_exp_and_others): sigmoid(u) =
  exp(-ln(1+exp(-u))), tanh(v) = 2*sigmoid(2v)-1, softplus(x) = ln(1+exp(x)).
"""

import numpy as np
import ml_dtypes

BF16NP = ml_dtypes.bfloat16

T = 512
BFULL = 128
NCORES = 8
B = BFULL // NCORES  # 16
YD, HD, ZD = 32, 512, 64
G = 3 * HD           # 1536
GT = G // 128        # 12 gate M-tiles
HT = HD // 128       # 4 h K-tiles
LOG2PI = float(np.log(2.0 * np.pi))

_CACHE = {}


def build_nc(t_steps, dbg=False):
    import concourse.bass as bass
    import concourse.tile as tile
    import concourse.mybir as mybir
    from concourse import bacc
    from concourse.bass import ds, ts
    from contextlib import ExitStack

    F32 = mybir.dt.float32
    BF = mybir.dt.float16
    AF = mybir.ActivationFunctionType
    OP = mybir.AluOpType
    ET = mybir.EngineType
    TT = t_steps

    nc = bacc.Bacc(debug=dbg)

    def din(name, shape, dt=BF):
        return nc.dram_tensor(name, shape, dt, kind="ExternalInput")

    def dout(name, shape, dt=F32):
        return nc.dram_tensor(name, shape, dt, kind="ExternalOutput")

    ybwd = din("ybwd", [(TT + 1) * 33, B])     # reversed y (transposed) + ones row
    epy = din("epy", [(TT + 1) * 128, B], F32)  # blk t: 0:64 eps, 64:96 y_prev, 96:128 pad
    yT = din("yT", [TT * 32, B], F32)          # y transposed (NLL)

    wb0a = din("wb0a", [33, G])
    wb0h = din("wb0h", [HD, G])
    wb1i = din("wb1i", [HD, G])
    wb1h = din("wb1h", [HD, G])
    wf0a = din("wf0a", [97, G])
    wf0b = din("wf0b", [32, G])
    wf0h = din("wf0h", [HD, G])
    wf1i = din("wf1i", [HD, G])
    wf1h = din("wf1h", [HD, G])
    wzp1z = din("wzp1z", [65, 128], F32)
    wzp1ob = din("wzp1ob", [HD, 128], F32)
    wzp2 = din("wzp2", [128, 128], F32)
    bzp2 = din("bzp2", [1, 128], F32)
    wzph = din("wzph", [128, 128], F32)
    bzph = din("bzph", [1, 128], F32)
    wyem = din("wyem", [HD, 64])
    byem = din("byem", [1, 64])
    wyes = din("wyes", [65, 32])
    wztr = din("wztr", [65, 128], F32)
    wzth = din("wzth", [128, 128], F32)
    bzth = din("bzth", [1, 128], F32)
    bxb0g = din("bxb0g", [128, 64], F32)
    bxb1 = din("bxb1", [128, 256], F32)
    bxf0g = din("bxf0g", [128, 64], F32)
    bxf1 = din("bxf1", [128, 256], F32)

    # o_post row-block tE+1: cols 0:16 pos_m(tE), 16:32 pos_s(tE), 32:48 z(tE),
    # rows 0:32 of cols 48:64 = ye_s(tE-1). Block 0 stays zero (z_prev(0)=0
    # for the prior epilogue, which reads z_prev(t)=z(t-1) from block t).
    o_post = dout("o_post", [(TT + 2) * 64, 64])
    o_prim = dout("o_prim", [TT * 64, B])
    o_pris = dout("o_pris", [TT * 64, B])
    o_allh = dout("o_allh", [TT * 128, 128])   # row t*128+p, col l*64+16k+b
    o_kn = dout("o_kn", [1, 2])

    s_ob = nc.dram_tensor("s_ob", [TT * 128, 64], F32)
    s_Pe = nc.dram_tensor("s_Pe", [(TT + 1) * 128, 32], F32)

    with tile.TileContext(nc) as tc, ExitStack() as _ctx:
        pw = _ctx.enter_context(tc.tile_pool(name="pw", bufs=1))
        pcar = _ctx.enter_context(tc.tile_pool(name="pcar", bufs=1))
        px = _ctx.enter_context(tc.tile_pool(name="px", bufs=3))
        pp2 = _ctx.enter_context(tc.tile_pool(name="pp2", bufs=2))
        psp = _ctx.enter_context(tc.tile_pool(name="psp", bufs=1, space="PSUM"))

        # ------------- resident weights -------------
        def wload(dram, dt=BF):
            t = pw.tile(list(dram.shape), dt, tag=dram.name, name=dram.name)
            nc.sync.dma_start(t[:], dram[:])
            return t

        W = {}
        for dram in [wb0a, wf0a, wf0b, byem, wyes]:
            W[dram.name] = wload(dram, BF)
        for dram in [wzp1z, wzp2, bzp2, wzph, bzph, wztr, wzth, bzth,
                     bxb0g, bxb1, bxf0g, bxf1]:
            W[dram.name] = wload(dram, F32)

        def wload_k(dram, dt=BF):
            out = []
            for k in range(HT):
                t = pw.tile([128, dram.shape[1]], dt,
                            tag=f"{dram.name}_{k}", name=f"{dram.name}_{k}")
                nc.sync.dma_start(t[:], dram[k * 128:(k + 1) * 128, :])
                out.append(t)
            return out

        Wb0h, Wb1i, Wb1h = wload_k(wb0h), wload_k(wb1i), wload_k(wb1h)
        Wf0h, Wf1i, Wf1h = wload_k(wf0h), wload_k(wf1i), wload_k(wf1h)
        Wzp1ob, Wyem = wload_k(wzp1ob, F32), wload_k(wyem)

        # ------------- carries & constants -------------
        def fixed(shape, dt, name):
            return pcar.tile(shape, dt, tag=name, name=name)

        h0bf = fixed([128, 64], F32, "h0bf")
        h0bb = fixed([128, 64], BF, "h0bb")
        h1bf = fixed([128, 64], F32, "h1bf")
        h1bb = fixed([128, 64], BF, "h1bb")
        hff = fixed([128, 128], F32, "hff")   # fwd h: cols 0:64 L0, 64:128 L1
        hfb = fixed([128, 128], BF, "hfb")
        xaA = fixed([97, B], BF, "xaA")
        xaB = fixed([97, B], BF, "xaB")
        xbA = fixed([32, B], BF, "xbA")
        xbB = fixed([32, B], BF, "xbB")
        zbA = fixed([65, B], F32, "zbA")
        zbB = fixed([65, B], F32, "zbB")
        ybwA = fixed([33, B], BF, "ybwA")
        ybwB = fixed([33, B], BF, "ybwB")
        ryb = fixed([65, B], BF, "ryb")
        ones16 = fixed([1, B], BF, "ones16")
        ones16f = fixed([1, B], F32, "ones16f")
        onesN = fixed([1, 512], F32, "onesN")
        halves64 = fixed([64, 1], F32, "halves64")
        ones32f = fixed([32, 1], F32, "ones32f")
        kacc = fixed([64, 1], F32, "kacc")
        nacc = fixed([32, 1], F32, "nacc")

        for tl in (h0bf, h0bb, h1bf, h1bb, hff, hfb,
                   xaA, xaB, xbA, xbB, zbA, zbB, ryb):
            nc.vector.memset(tl[:], 0.0)
        for tl in (xaA, xaB):
            nc.vector.memset(tl[96:97, :], 1.0)
        for tl in (zbA, zbB, ryb):
            nc.vector.memset(tl[64:65, :], 1.0)
        nc.vector.memset(ones16[:], 1.0)
        nc.vector.memset(ones16f[:], 1.0)
        nc.vector.memset(onesN[:], 1.0)
        nc.vector.memset(halves64[:], 0.5)
        nc.vector.memset(ones32f[:], 1.0)
        nc.vector.memset(kacc[:], 0.0)
        nc.vector.memset(nacc[:], 0.0)

        zP = px.tile([128, B], F32, tag="zPpad", name="zPpad")
        nc.vector.memset(zP[:], 0.0)
        nc.sync.dma_start(s_Pe[TT * 128:(TT + 1) * 128, 0:16], zP[:])
        EPC = (TT + 1 + 3) // 4
        for h in range(4):
            lo = h * EPC
            hi = min((h + 1) * EPC, TT + 1)
            if lo >= hi:
                continue
            nc.sync.dma_start(
                s_Pe[lo * 128:hi * 128, 16:32].rearrange("(t p) c -> t p c", p=128),
                epy[lo * 128:hi * 128, :].rearrange("(t f) b -> t f b", f=128))

        # ------------- helpers -------------
        def bank_mms(mm_list):
            """Emit one PSUM accumulation group (one bank). List order is the
            intended execution order; the first MM (start=True, whole-bank
            has_written clear) is forced to execute first and the last
            (stop=True) last via scheduler deps."""
            n = len(mm_list)
            first_inst = None
            insts = []
            for i, (o, l, r) in enumerate(mm_list):
                inst = nc.tensor.matmul(o, l, r, start=(i == 0), stop=(i == n - 1))
                if i > 0:
                    tile.add_dep_helper(inst.ins, first_inst.ins, sync=False,
                                        reason="bank start order")
                if i == n - 1 and n > 2:
                    for other in insts[1:]:
                        tile.add_dep_helper(inst.ins, other.ins, sync=False,
                                            reason="bank stop order")
                if i == 0:
                    first_inst = inst
                insts.append(inst)
            return insts

        def cell(prz, pgin, pghn, hf, hb, b_rz, b_gin, b_ghn):
            """GRU cell nonlinearity; hf/hb are [128,64] APs (fp32/bf16)."""
            if b_rz is not None:
                u = px.tile([128, 128], F32, tag="c_u", name="c_u")
                nc.vector.tensor_tensor(u[:], prz[:], b_rz, OP.add)
                src = u[:]
            else:
                src = prz[:]
            e1 = px.tile([128, 128], F32, tag="c_e1", name="c_e1")
            nc.scalar.activation(e1[:], src, AF.Exp, scale=-1.0)
            l1 = px.tile([128, 128], F32, tag="c_l1", name="c_l1")
            nc.scalar.activation(l1[:], e1[:], AF.Ln, bias=1.0)
            s = px.tile([128, 128], F32, tag="c_s", name="c_s")
            nc.scalar.activation(s[:], l1[:], AF.Exp, scale=-1.0)
            t1 = px.tile([128, 64], F32, tag="c_t1", name="c_t1")
            nc.vector.tensor_tensor(t1[:], pghn[:], b_ghn, OP.add)
            t2 = px.tile([128, 64], F32, tag="c_t2", name="c_t2")
            nc.vector.tensor_tensor(t2[:], s[:, 0:64], t1[:], OP.mult)
            if b_gin is not None:
                t3a = px.tile([128, 64], F32, tag="c_t3a", name="c_t3a")
                nc.vector.tensor_tensor(t3a[:], pgin[:], b_gin, OP.add)
                gin_ap = t3a[:]
            else:
                gin_ap = pgin[:]
            t3 = px.tile([128, 64], F32, tag="c_t3", name="c_t3")
            nc.vector.tensor_tensor(t3[:], t2[:], gin_ap, OP.add)
            e2 = px.tile([128, 64], F32, tag="c_e2", name="c_e2")
            nc.scalar.activation(e2[:], t3[:], AF.Exp, scale=-2.0)
            l2 = px.tile([128, 64], F32, tag="c_l2", name="c_l2")
            nc.scalar.activation(l2[:], e2[:], AF.Ln, bias=1.0)
            sn = px.tile([128, 64], F32, tag="c_sn", name="c_sn")
            nc.scalar.activation(sn[:], l2[:], AF.Exp, scale=-1.0)
            nt = px.tile([128, 64], F32, tag="c_nt", name="c_nt")
            nc.vector.tensor_scalar(nt[:], sn[:], 2.0, -1.0, OP.mult, OP.add)
            d = px.tile([128, 64], F32, tag="c_d", name="c_d")
            nc.vector.tensor_tensor(d[:], hf, nt[:], OP.subtract)
            zd = px.tile([128, 64], F32, tag="c_zd", name="c_zd")
            nc.vector.tensor_tensor(zd[:], s[:, 64:128], d[:], OP.mult)
            nc.vector.tensor_tensor(hf, nt[:], zd[:], OP.add)
            nc.vector.tensor_copy(hb, hf)

        def posterior(tE, zb_cur, zb_nxt, xa_nxt, opost):
            """Posterior/sample for step tE; writes opost cols 0:48, DMAs it."""
            pPe = pp2.tile([128, 32], F32, tag="pPe", name="pPe")
            nc.sync.dma_start(pPe[:], s_Pe[ds(tE * 128, 128)])
            nc.vector.tensor_copy(xa_nxt[0:32, :], pPe[64:96, 16:32])
            q1 = psp.tile([128, B], F32, tag="psmall", name="q1", bufs=3)
            nc.tensor.matmul(q1[:], W["wzp1z"][:], zb_cur[:], start=True, stop=True)
            u1 = px.tile([128, B], F32, tag="p_u1", name="p_u1")
            nc.vector.tensor_tensor(u1[:], q1[:], pPe[:, 0:16], OP.add)
            r1 = px.tile([128, B], F32, tag="p_r1", name="p_r1")
            nc.scalar.activation(r1[:], u1[:], AF.Relu)
            q2 = psp.tile([128, B], F32, tag="psmall", name="q2", bufs=3)
            bank_mms([(q2[:], W["bzp2"][:], ones16f[:]),
                      (q2[:], W["wzp2"][:], r1[:])])
            r2 = px.tile([128, B], F32, tag="p_r2", name="p_r2")
            nc.scalar.activation(r2[:], q2[:], AF.Relu)
            ph = psp.tile([128, B], F32, tag="psmall", name="ph", bufs=3)
            bank_mms([(ph[:], W["bzph"][:], ones16f[:]),
                      (ph[:], W["wzph"][:], r2[:])])
            nc.vector.tensor_copy(opost[:, 0:16], ph[0:64, :])          # pos_m
            eS = px.tile([64, B], F32, tag="p_es", name="p_es")
            nc.scalar.activation(eS[:], ph[64:128, :], AF.Exp)
            nc.scalar.activation(opost[:, 16:32], eS[:], AF.Ln, bias=1.0)  # pos_s
            z1 = px.tile([64, B], F32, tag="p_z1", name="p_z1")
            nc.vector.tensor_tensor(z1[:], pPe[0:64, 16:32], opost[:, 16:32], OP.mult)
            nc.vector.tensor_tensor(opost[:, 32:48], z1[:], opost[:, 0:16], OP.add)
            nc.vector.tensor_copy(zb_nxt[0:64, :], opost[:, 32:48])
            nc.vector.tensor_copy(xa_nxt[32:64, :], opost[0:32, 32:48])
            nc.vector.tensor_copy(xa_nxt[64:96, :], opost[32:64, 32:48])
            nc.sync.dma_start(o_post[ds((tE + 1) * 64, 64)], opost[:])

        def emission(h1b, opost):
            """Emission std for current step t; writes opost[0:32, 48:64]."""
            y1 = psp.tile([64, B], F32, tag="psmall", name="y1", bufs=3)
            bank_mms([(y1[:], W["byem"][:], ones16[:])] +
                     [(y1[:], Wyem[k][:], h1b[:, 16 * k:16 * k + 16])
                      for k in range(HT)])
            ry = px.tile([64, B], F32, tag="e_ry", name="e_ry")
            nc.scalar.activation(ry[:], y1[:], AF.Relu)
            nc.vector.tensor_copy(ryb[0:64, :], ry[:])
            y2 = psp.tile([32, B], F32, tag="psmall", name="y2", bufs=3)
            nc.tensor.matmul(y2[:], W["wyes"][:], ryb[:], start=True, stop=True)
            eY = px.tile([32, B], F32, tag="e_ey", name="e_ey")
            nc.scalar.activation(eY[:], y2[:], AF.Exp)
            nc.scalar.activation(opost[0:32, 48:64], eY[:], AF.Ln, bias=1.0)
            return opost[0:32, 48:64]

        def gate_banks(name):
            prz = psp.tile([128, 128], F32, tag="pgate", name=name + "rz", bufs=4)
            pgin = psp.tile([128, 64], F32, tag="pgate", name=name + "gi", bufs=4)
            pghn = psp.tile([128, 64], F32, tag="pgate", name=name + "gh", bufs=4)
            return prz, pgin, pghn

        def rzs(ps, j):
            return ps[:, 16 * j:16 * j + 16]

        def ns(ps, j):
            return ps[:, 16 * (j - 8):16 * (j - 8) + 16]

        def bwd_step(t, cur_ybw, nxt_ybw):
            nc.sync.dma_start(nxt_ybw[:], ybwd[ds((t + 1) * 33, 33)])
            prz, pgin, pghn = gate_banks("b0")
            bank_mms([(rzs(prz, j), W["wb0a"][:, ts(j, 128)], cur_ybw[:])
                      for j in range(8)] +
                     [(rzs(prz, j), Wb0h[k][:, ts(j, 128)], h0bb[:, ts(k, 16)])
                      for k in range(HT) for j in range(8)])
            bank_mms([(ns(pgin, j), W["wb0a"][:, ts(j, 128)], cur_ybw[:])
                      for j in range(8, 12)])
            bank_mms([(ns(pghn, j), Wb0h[k][:, ts(j, 128)], h0bb[:, ts(k, 16)])
                      for k in range(HT) for j in range(8, 12)])
            cell(prz, pgin, pghn, h0bf[:], h0bb[:], None, None, W["bxb0g"][:])
            prz, pgin, pghn = gate_banks("b1")
            bank_mms([(rzs(prz, j), Wb1h[k][:, ts(j, 128)], h1bb[:, ts(k, 16)])
                      for k in range(HT) for j in range(8)] +
                     [(rzs(prz, j), Wb1i[k][:, ts(j, 128)], h0bb[:, ts(k, 16)])
                      for k in range(HT) for j in range(8)])
            bank_mms([(ns(pgin, j), Wb1i[k][:, ts(j, 128)], h0bb[:, ts(k, 16)])
                      for k in range(HT) for j in range(8, 12)])
            bank_mms([(ns(pghn, j), Wb1h[k][:, ts(j, 128)], h1bb[:, ts(k, 16)])
                      for k in range(HT) for j in range(8, 12)])
            cell(prz, pgin, pghn, h1bf[:], h1bb[:], W["bxb1"][:, 0:128],
                 W["bxb1"][:, 128:192], W["bxb1"][:, 192:256])
            nc.sync.dma_start(s_ob[ds((TT - 1 - t) * 128, 128)], h1bf[:])

        def fwd_step(t, xa_c, xb_c, zb_c, xa_n, xb_n, zb_n):
            opost = px.tile([64, 64], F32, tag="opost", name="opost")
            nc.vector.memset(opost[32:64, 48:64], 0.0)
            prz, pgin, pghn = gate_banks("f0")
            bank_mms([(rzs(prz, j), Wf0h[k][:, ts(j, 128)],
                       hfb[:, 16 * k:16 * k + 16])
                      for k in range(HT) for j in range(8)] +
                     [(rzs(prz, j), W["wf0a"][:, ts(j, 128)], xa_c[:])
                      for j in range(8)] +
                     [(rzs(prz, j), W["wf0b"][:, ts(j, 128)], xb_c[:])
                      for j in range(8)])
            bank_mms([(ns(pgin, j), W["wf0a"][:, ts(j, 128)], xa_c[:])
                      for j in range(8, 12)] +
                     [(ns(pgin, j), W["wf0b"][:, ts(j, 128)], xb_c[:])
                      for j in range(8, 12)])
            bank_mms([(ns(pghn, j), Wf0h[k][:, ts(j, 128)],
                       hfb[:, 16 * k:16 * k + 16])
                      for k in range(HT) for j in range(8, 12)])
            cell(prz, pgin, pghn, hff[:, 0:64], hfb[:, 0:64], None, None,
                 W["bxf0g"][:])
            prz, pgin, pghn = gate_banks("f1")
            bank_mms([(rzs(prz, j), Wf1h[k][:, ts(j, 128)],
                       hfb[:, 64 + 16 * k:64 + 16 * k + 16])
                      for k in range(HT) for j in range(8)] +
                     [(rzs(prz, j), Wf1i[k][:, ts(j, 128)],
                       hfb[:, 16 * k:16 * k + 16])
                      for k in range(HT) for j in range(8)])
            bank_mms([(ns(pgin, j), Wf1i[k][:, ts(j, 128)],
                       hfb[:, 16 * k:16 * k + 16])
                      for k in range(HT) for j in range(8, 12)])
            bank_mms([(ns(pghn, j), Wf1h[k][:, ts(j, 128)],
                       hfb[:, 64 + 16 * k:64 + 16 * k + 16])
                      for k in range(HT) for j in range(8, 12)])
            cell(prz, pgin, pghn, hff[:, 64:128], hfb[:, 64:128],
                 W["bxf1"][:, 0:128], W["bxf1"][:, 128:192],
                 W["bxf1"][:, 192:256])
            nc.sync.dma_start(o_allh[ds(t * 128, 128)], hff[:])
            ys = emission(hfb[:, 64:128], opost)
            nc.vector.tensor_copy(xb_n[:], ys)
            posterior(t + 1, zb_c, zb_n, xa_n, opost)

        # ================= phase 1: backward GRU =================
        nc.sync.dma_start(ybwA[:], ybwd[0:33, :])
        UNB = 2
        with tc.For_i(0, TT, UNB, hint_engines=(ET.PE,)) as t4:
            for u in range(UNB):
                cur, nxt = (ybwA, ybwB) if u % 2 == 0 else (ybwB, ybwA)
                bwd_step(t4 + u, cur, nxt)

        # ================= phase 2: P_ob batched =================
        CP = min(8, TT)
        assert TT % CP == 0
        for c in range(TT // CP):
            obt = pp2.tile([128, CP * 64], F32, tag="obt", name="obt")
            nc.sync.dma_start(
                obt[:].rearrange("p (t c) -> p t c", t=CP),
                s_ob[c * CP * 128:(c + 1) * CP * 128, :].rearrange(
                    "(t p) c -> p t c", p=128))
            psP = psp.tile([128, CP * 16], F32, tag="pgate", name="psP", bufs=4)
            obt3 = obt[:].rearrange("p (t c) -> p t c", t=CP)
            bank_mms([(psP[:], Wzp1ob[k][:], obt3[:, :, 16 * k:16 * k + 16])
                      for k in range(HT)])
            sbP = pp2.tile([128, CP * 16], F32, tag="sbP", name="sbP")
            nc.vector.tensor_copy(sbP[:], psP[:])
            nc.sync.dma_start(
                s_Pe[c * CP * 128:(c + 1) * CP * 128, 0:16].rearrange(
                    "(t p) c -> p t c", p=128),
                sbP[:].rearrange("p (t c) -> p t c", t=CP))

        # ================= phase 3: forward scan =================
        opost0 = px.tile([64, 64], F32, tag="opost", name="opost0")
        nc.vector.memset(opost0[:, 48:64], 0.0)
        posterior(0, zbA, zbA, xaA, opost0)
        UNF = 2
        with tc.For_i(0, TT, UNF, hint_engines=(ET.PE,)) as t2:
            for u in range(UNF):
                if u % 2 == 0:
                    fwd_step(t2 + u, xaA, xbA, zbA, xaB, xbB, zbB)
                else:
                    fwd_step(t2 + u, xaB, xbB, zbB, xaA, xbA, zbA)

        # ================= phase 4: prior + KLD/NLL epilogue =================
        CK = min(16, TT)
        assert TT % CK == 0
        N = CK * B
        for c in range(TT // CK):
            zpf = pp2.tile([64, N], F32, tag="zpf", name="zpf")
            nc.sync.dma_start(
                zpf[:].rearrange("f (t b) -> f t b", t=CK),
                o_post[c * CK * 64:(c + 1) * CK * 64, 32:48].rearrange(
                    "(t f) b -> f t b", f=64))
            zpt = pp2.tile([65, N], F32, tag="zpt", name="zpt")
            nc.vector.tensor_copy(zpt[0:64, :], zpf[:])
            nc.vector.memset(zpt[64:65, :], 1.0)
            pzt = psp.tile([128, N], F32, tag="pgate", name="pzt", bufs=4)
            nc.tensor.matmul(pzt[:], W["wztr"][:], zpt[:], start=True, stop=True)
            rq = pp2.tile([128, N], F32, tag="rq", name="rq")
            nc.scalar.activation(rq[:], pzt[:], AF.Relu)
            phh = psp.tile([128, N], F32, tag="pgate", name="phh", bufs=4)
            bank_mms([(phh[:], W["bzth"][:], onesN[:, 0:N]),
                      (phh[:], W["wzth"][:], rq[:])])
            prm = pp2.tile([64, N], F32, tag="prm", name="prm")
            nc.vector.tensor_copy(prm[:], phh[0:64, :])
            nc.sync.dma_start(
                o_prim[c * CK * 64:(c + 1) * CK * 64, :].rearrange(
                    "(t f) b -> f t b", f=64),
                prm[:].rearrange("f (t b) -> f t b", t=CK))
            pre = pp2.tile([64, N], F32, tag="pre", name="pre")
            nc.scalar.activation(pre[:], phh[64:128, :], AF.Exp)
            prs = pp2.tile([64, N], F32, tag="prs", name="prs")
            nc.scalar.activation(prs[:], pre[:], AF.Ln, bias=1.0)
            nc.sync.dma_start(
                o_pris[c * CK * 64:(c + 1) * CK * 64, :].rearrange(
                    "(t f) b -> f t b", f=64),
                prs[:].rearrange("f (t b) -> f t b", t=CK))
            # KLD terms (pos_m/pos_s read back from o_post)
            pom = pp2.tile([64, N], F32, tag="pom", name="pom")
            nc.sync.dma_start(
                pom[:].rearrange("f (t b) -> f t b", t=CK),
                o_post[(c * CK + 1) * 64:((c + 1) * CK + 1) * 64, 0:16].rearrange(
                    "(t f) b -> f t b", f=64))
            pss = pp2.tile([64, N], F32, tag="pss", name="pss")
            nc.sync.dma_start(
                pss[:].rearrange("f (t b) -> f t b", t=CK),
                o_post[(c * CK + 1) * 64:((c + 1) * CK + 1) * 64, 16:32].rearrange(
                    "(t f) b -> f t b", f=64))
            dm = pp2.tile([64, N], F32, tag="dm", name="dm")
            nc.vector.tensor_tensor(dm[:], pom[:], prm[:], OP.subtract)
            dm2 = pp2.tile([64, N], F32, tag="dm2", name="dm2")
            nc.scalar.activation(dm2[:], dm[:], AF.Square)
            ps2 = pp2.tile([64, N], F32, tag="ps2", name="ps2")
            nc.scalar.activation(ps2[:], pss[:], AF.Square)
            lp = pp2.tile([64, N], F32, tag="lp", name="lp")
            nc.scalar.activation(lp[:], prs[:], AF.Ln)
            ls = pp2.tile([64, N], F32, tag="ls", name="ls")
            nc.scalar.activation(ls[:], pss[:], AF.Ln)
            iv = pp2.tile([64, N], F32, tag="iv", name="iv")
            nc.scalar.activation(iv[:], lp[:], AF.Exp, scale=-2.0)
            num = pp2.tile([64, N], F32, tag="num", name="num")
            nc.vector.tensor_tensor(num[:], ps2[:], dm2[:], OP.add)
            qq = pp2.tile([64, N], F32, tag="qq", name="qq")
            nc.vector.tensor_tensor(qq[:], num[:], iv[:], OP.mult)
            ad = pp2.tile([64, N], F32, tag="ad", name="ad")
            nc.vector.tensor_tensor(ad[:], lp[:], ls[:], OP.subtract)
            ad2 = pp2.tile([64, N], F32, tag="ad2", name="ad2")
            nc.vector.tensor_scalar(ad2[:], ad[:], 2.0, -1.0, OP.mult, OP.add)
            term = pp2.tile([64, N], F32, tag="term", name="term")
            nc.vector.tensor_tensor(term[:], qq[:], ad2[:], OP.add)
            red = pp2.tile([64, 1], F32, tag="red", name="red")
            nc.vector.tensor_reduce(red[:], term[:], mybir.AxisListType.X, OP.add)
            nc.vector.tensor_tensor(kacc[:], kacc[:], red[:], OP.add)
            # NLL terms (ye_s(t) lives at o_post block t+1, cols 48:64)
            ych = pp2.tile([32, N], F32, tag="ych", name="ych")
            nc.sync.dma_start(
                ych[:].rearrange("f (t b) -> f t b", t=CK),
                yT[c * CK * 32:(c + 1) * CK * 32, :].rearrange(
                    "(t f) b -> f t b", f=32))
            ysc = pp2.tile([32, N], F32, tag="ysc", name="ysc")
            nc.sync.dma_start(
                ysc[:].rearrange("f (t b) -> f t b", t=CK),
                o_post[(c * CK + 2) * 64:((c + 1) * CK + 2) * 64, 48:64].rearrange(
                    "(t f) b -> f t b", f=64)[0:32])
            ly = pp2.tile([32, N], F32, tag="ly", name="ly")
            nc.scalar.activation(ly[:], ysc[:], AF.Ln)
            ivy = pp2.tile([32, N], F32, tag="ivy", name="ivy")
            nc.scalar.activation(ivy[:], ly[:], AF.Exp, scale=-2.0)
            y2 = pp2.tile([32, N], F32, tag="y2", name="y2")
            nc.scalar.activation(y2[:], ych[:], AF.Square)
            t5 = pp2.tile([32, N], F32, tag="t5", name="t5")
            nc.vector.tensor_tensor(t5[:], y2[:], ivy[:], OP.mult)
            t6 = pp2.tile([32, N], F32, tag="t6", name="t6")
            nc.vector.tensor_scalar(t6[:], t5[:], 0.5, 0.5 * LOG2PI, OP.mult, OP.add)
            t7 = pp2.tile([32, N], F32, tag="t7", name="t7")
            nc.vector.tensor_tensor(t7[:], t6[:], ly[:], OP.add)
            redn = pp2.tile([32, 1], F32, tag="redn", name="redn")
            nc.vector.tensor_reduce(redn[:], t7[:], mybir.AxisListType.X, OP.add)
            nc.vector.tensor_tensor(nacc[:], nacc[:], redn[:], OP.add)

        pk1 = psp.tile([1, 1], F32, tag="psmall", name="pk1", bufs=3)
        pk2 = psp.tile([1, 1], F32, tag="psmall", name="pk2", bufs=3)
        nc.tensor.matmul(pk1[:], halves64[:], kacc[:], start=True, stop=True)
        nc.tensor.matmul(pk2[:], ones32f[:], nacc[:], start=True, stop=True)
        skn = pp2.tile([1, 2], F32, tag="skn", name="skn")
        nc.vector.tensor_copy(skn[:, 0:1], pk1[:])
        nc.vector.tensor_copy(skn[:, 1:2], pk2[:])
        nc.sync.dma_start(o_kn[:], skn[:])

    nc.compile()
    return nc


# ==================== host-side prep ====================

def _lhsT(w):
    return np.ascontiguousarray(np.asarray(w, np.float32).T)


def _bf(a):
    return np.ascontiguousarray(np.asarray(a, np.float32)).astype(np.float16)


def _expand_bias(b, ncols):
    F = b.shape[0]
    jt = F // 128
    out = np.zeros((128, jt * 16), np.float32)
    for j in range(jt):
        out[:, 16 * j:16 * j + 16] = b[128 * j:128 * j + 128, None]
    assert jt * 16 == ncols
    return out


def prep_weights(inp):
    g = lambda k: np.asarray(inp[k], np.float32)
    w = {}

    def gru_bias(bih, bhh):
        brz = bih[0:2 * HD] + bhh[0:2 * HD]
        row = np.concatenate([brz, bih[2 * HD:]])
        ghn = _expand_bias(bhh[2 * HD:], 64)
        full = np.concatenate([_expand_bias(brz, 128),
                               _expand_bias(bih[2 * HD:], 64), ghn], axis=1)
        return row, ghn, full

    row_b0, ghn_b0, _ = gru_bias(g("bwd_bih0"), g("bwd_bhh0"))
    w["wb0a"] = _bf(np.concatenate([_lhsT(g("bwd_Wih0")), row_b0[None, :]], 0))
    w["wb0h"] = _bf(_lhsT(g("bwd_Whh0")))
    w["bxb0g"] = ghn_b0
    row_b1, _, full_b1 = gru_bias(g("bwd_bih1"), g("bwd_bhh1"))
    w["wb1i"] = _bf(_lhsT(g("bwd_Wih1")))
    w["wb1h"] = _bf(_lhsT(g("bwd_Whh1")))
    w["bxb1"] = full_b1
    row_f0, ghn_f0, _ = gru_bias(g("fwd_bih0"), g("fwd_bhh0"))
    wt = _lhsT(g("fwd_Wih0"))
    w["wf0a"] = _bf(np.concatenate([wt[0:96], row_f0[None, :]], 0))
    w["wf0b"] = _bf(wt[96:128])
    w["wf0h"] = _bf(_lhsT(g("fwd_Whh0")))
    row_f1, _, full_f1 = gru_bias(g("fwd_bih1"), g("fwd_bhh1"))
    w["wf1i"] = _bf(_lhsT(g("fwd_Wih1")))
    w["wf1h"] = _bf(_lhsT(g("fwd_Whh1")))
    w["bxf0g"] = ghn_f0
    w["bxf1"] = full_f1
    z1 = _lhsT(g("zp_W1"))
    w["wzp1ob"] = z1[0:HD]
    w["wzp1z"] = np.ascontiguousarray(
        np.concatenate([z1[HD:HD + ZD], g("zp_b1")[None, :]], 0))
    w["wzp2"] = _lhsT(g("zp_W2"))
    w["bzp2"] = np.ascontiguousarray(g("zp_b2")[None, :])
    w["wzph"] = np.ascontiguousarray(
        np.concatenate([_lhsT(g("zp_mean_W")), _lhsT(g("zp_std_W"))], 1))
    w["bzph"] = np.ascontiguousarray(
        np.concatenate([g("zp_mean_b"), g("zp_std_b")])[None, :])
    w["wyem"] = _bf(_lhsT(g("yem_W")))
    w["byem"] = _bf(g("yem_b")[None, :])
    w["wyes"] = _bf(np.concatenate([_lhsT(g("yem_std_W")), g("yem_std_b")[None, :]], 0))
    w["wztr"] = np.ascontiguousarray(
        np.concatenate([_lhsT(g("ztr_W")), g("ztr_b")[None, :]], 0))
    w["wzth"] = np.ascontiguousarray(
        np.concatenate([_lhsT(g("ztr_mean_W")), _lhsT(g("ztr_std_W"))], 1))
    w["bzth"] = np.ascontiguousarray(
        np.concatenate([g("ztr_mean_b"), g("ztr_std_b")])[None, :])
    return w


def prep_core_inputs(y, eps, wshared, t_steps):
    TT = t_steps
    Bc = y.shape[1]
    m = dict(wshared)
    yr = y[::-1]
    ybwd = np.zeros((TT + 1, 33, Bc), np.float32)
    ybwd[0:TT, 0:32] = yr.transpose(0, 2, 1)
    ybwd[:, 32] = 1.0
    m["ybwd"] = _bf(ybwd.reshape((TT + 1) * 33, Bc))
    epy = np.zeros((TT + 1, 128, Bc), np.float32)
    epy[0:TT, 0:64] = eps.transpose(0, 2, 1)
    epy[1:TT, 64:96] = y[0:TT - 1].transpose(0, 2, 1)
    m["epy"] = np.ascontiguousarray(epy.reshape((TT + 1) * 128, Bc))
    m["yT"] = np.ascontiguousarray(
        y.transpose(0, 2, 1).reshape(TT * 32, Bc).astype(np.float32))
    return m


def unpack_outputs(res, t_steps, Bc):
    TT = t_steps

    def tb(a2, F):  # [(T)*F, Bc] -> [T, Bc, F]
        return a2.reshape(-1, F, Bc).transpose(0, 2, 1)

    po = res["o_post"].reshape(TT + 2, 64, 64)  # [blk, part, col]
    out = {}
    out["pos_m"] = po[1:TT + 1, :, 0:16].transpose(0, 2, 1)
    out["pos_s"] = po[1:TT + 1, :, 16:32].transpose(0, 2, 1)
    out["z"] = po[1:TT + 1, :, 32:48].transpose(0, 2, 1)
    out["ye_s"] = po[2:TT + 2, 0:32, 48:64].transpose(0, 2, 1)
    out["pri_m"] = tb(res["o_prim"], 64)
    out["pri_s"] = tb(res["o_pris"], 64)
    ah = res["o_allh"].reshape(TT, 128, 2, HT, Bc)  # [t, p, l, k, b]
    out["all_h"] = ah.transpose(0, 2, 4, 3, 1).reshape(TT, 2, Bc, HD)
    out["kld"] = float(res["o_kn"][0, 0])
    out["nll"] = float(res["o_kn"][0, 1])
    return out


def _get_nc(t_steps):
    key = ("nc", t_steps)
    if key not in _CACHE:
        _CACHE[key] = build_nc(t_steps)
    return _CACHE[key]


def make_in_maps(inp, t_steps, cores):
    y = np.asarray(inp["y"], np.float32)
    eps = np.asarray(inp["eps"], np.float32)
    Bc = y.shape[1] // cores
    wshared = prep_weights(inp)
    return [prep_core_inputs(y[:, c * Bc:(c + 1) * Bc],
                             eps[:, c * Bc:(c + 1) * Bc], wshared, t_steps)
            for c in range(cores)], Bc


def assemble(results, t_steps, Bc):
    outs = [unpack_outputs(res, t_steps, Bc) for res in results]
    kld = np.float32(sum(o["kld"] for o in outs))
    nll = np.float32(sum(o["nll"] for o in outs))

    def cat(k):
        return np.ascontiguousarray(
            np.concatenate([o[k] for o in outs], axis=-2).astype(np.float32))

    return (kld, nll, cat("pos_m"), cat("pos_s"), cat("pri_m"), cat("pri_s"),
            cat("z"), cat("ye_s"), cat("all_h"))


def run_cores(inp, t_steps=T, cores=NCORES):
    from concourse.bass_utils import run_bass_kernel_spmd
    in_maps, Bc = make_in_maps(inp, t_steps, cores)
    nc = _get_nc(t_steps)
    r = run_bass_kernel_spmd(nc, in_maps, core_ids=list(range(cores)))
    return assemble(r.results, t_steps, Bc)


def kernel(**inputs):
    return run_cores(inputs, t_steps=T, cores=NCORES)


# revision 28
# speedup vs baseline: 97.5787x; 97.5787x over previous
"""Trainium2 Bass kernel for nn_DSVM (deep state-space variational model).

Strategy (self-contained; shapes hardcoded):
- 8-way data-parallel over batch B=128 -> 16 samples/core, SPMD, no
  collectives. Scalar losses come back as per-core partials summed on host.
- Transposed compute layout everywhere: [features, batch] with features on
  SBUF partitions. Weights are host-pre-transposed into lhsT [K_in, M_out]
  bf16 tiles and stay SBUF-resident for all 512 steps; activations stream as
  the moving operand.
- Sequential phases: (1) backward 2-layer GRU over reversed y, storing the
  top-layer output ob_t; (2) batched matmul P_ob = zp_W1[:, :H] @ ob_t for
  all t; (3) forward scan: posterior MLP (pipelined one step ahead),
  2-layer GRU, emission net, with gi+gh fused via PSUM accumulation and all
  biases folded into ones-row matmul tricks / precomputed fp32 bias tiles;
  (4) batched epilogue: prior MLP from stored z_prev, KLD/NLL reductions.
- Single ACT table set (natural_log_exp_and_others): sigmoid(u) =
  exp(-ln(1+exp(-u))), tanh(v) = 2*sigmoid(2v)-1, softplus(x) = ln(1+exp(x)).
"""

import numpy as np
import ml_dtypes

BF16NP = ml_dtypes.bfloat16

T = 512
BFULL = 128
NCORES = 8
B = BFULL // NCORES  # 16
YD, HD, ZD = 32, 512, 64
G = 3 * HD           # 1536
GT = G // 128        # 12 gate M-tiles
HT = HD // 128       # 4 h K-tiles
LOG2PI = float(np.log(2.0 * np.pi))

_CACHE = {}


def build_nc(t_steps, dbg=False):
    import concourse.bass as bass
    import concourse.tile as tile
    import concourse.mybir as mybir
    from concourse import bacc
    from concourse.bass import ds, ts
    from contextlib import ExitStack

    F32 = mybir.dt.float32
    BF = mybir.dt.float16
    AF = mybir.ActivationFunctionType
    OP = mybir.AluOpType
    ET = mybir.EngineType
    TT = t_steps

    nc = bacc.Bacc(debug=dbg)

    def din(name, shape, dt=BF):
        return nc.dram_tensor(name, shape, dt, kind="ExternalInput")

    def dout(name, shape, dt=F32):
        return nc.dram_tensor(name, shape, dt, kind="ExternalOutput")

    ybwd = din("ybwd", [(TT + 1) * 33, B])     # reversed y (transposed) + ones row
    epy = din("epy", [(TT + 1) * 128, B], F32)  # blk t: 0:64 eps, 64:96 y_prev, 96:128 pad
    yT = din("yT", [TT * 32, B], F32)          # y transposed (NLL)

    wb0a = din("wb0a", [33, G])
    wb0h = din("wb0h", [HD, G])
    wb1i = din("wb1i", [HD, G])
    wb1h = din("wb1h", [HD, G])
    wf0a = din("wf0a", [97, G])
    wf0b = din("wf0b", [32, G])
    wf0h = din("wf0h", [HD, G])
    wf1i = din("wf1i", [HD, G])
    wf1h = din("wf1h", [HD, G])
    wzp1z = din("wzp1z", [65, 128], F32)
    wzp1ob = din("wzp1ob", [HD, 128], F32)
    wzp2 = din("wzp2", [128, 128], F32)
    bzp2 = din("bzp2", [1, 128], F32)
    wzph = din("wzph", [128, 128], F32)
    bzph = din("bzph", [1, 128], F32)
    wyem = din("wyem", [HD, 64])
    byem = din("byem", [1, 64])
    wyes = din("wyes", [65, 32])
    wztr = din("wztr", [65, 128], F32)
    wzth = din("wzth", [128, 128], F32)
    bzth = din("bzth", [1, 128], F32)
    bxb0g = din("bxb0g", [128, 64], F32)
    bxb1 = din("bxb1", [128, 256], F32)
    bxf0g = din("bxf0g", [128, 64], F32)
    bxf1 = din("bxf1", [128, 256], F32)

    # o_post row-block tE+1: cols 0:16 pos_m(tE), 16:32 pos_s(tE), 32:48 z(tE),
    # rows 0:32 of cols 48:64 = ye_s(tE-1). Block 0 stays zero (z_prev(0)=0
    # for the prior epilogue, which reads z_prev(t)=z(t-1) from block t).
    o_post = dout("o_post", [(TT + 2) * 64, 64])
    o_prim = dout("o_prim", [TT * 64, B])
    o_pris = dout("o_pris", [TT * 64, B])
    o_allh = dout("o_allh", [TT * 128, 128])   # row t*128+p, col l*64+16k+b
    o_kn = dout("o_kn", [1, 2])

    s_ob = nc.dram_tensor("s_ob", [TT * 128, 64], F32)
    s_Pe = nc.dram_tensor("s_Pe", [(TT + 1) * 128, 32], F32)

    with tile.TileContext(nc) as tc, ExitStack() as _ctx:
        pw = _ctx.enter_context(tc.tile_pool(name="pw", bufs=1))
        pcar = _ctx.enter_context(tc.tile_pool(name="pcar", bufs=1))
        px = _ctx.enter_context(tc.tile_pool(name="px", bufs=3))
        pp2 = _ctx.enter_context(tc.tile_pool(name="pp2", bufs=2))
        psp = _ctx.enter_context(tc.tile_pool(name="psp", bufs=1, space="PSUM"))

        # ------------- resident weights -------------
        def wload(dram, dt=BF):
            t = pw.tile(list(dram.shape), dt, tag=dram.name, name=dram.name)
            nc.sync.dma_start(t[:], dram[:])
            return t

        W = {}
        for dram in [wb0a, wf0a, wf0b, byem, wyes]:
            W[dram.name] = wload(dram, BF)
        for dram in [wzp1z, wzp2, bzp2, wzph, bzph, wztr, wzth, bzth,
                     bxb0g, bxb1, bxf0g, bxf1]:
            W[dram.name] = wload(dram, F32)

        def wload_k(dram, dt=BF):
            out = []
            for k in range(HT):
                t = pw.tile([128, dram.shape[1]], dt,
                            tag=f"{dram.name}_{k}", name=f"{dram.name}_{k}")
                nc.sync.dma_start(t[:], dram[k * 128:(k + 1) * 128, :])
                out.append(t)
            return out

        Wb0h, Wb1i, Wb1h = wload_k(wb0h), wload_k(wb1i), wload_k(wb1h)
        Wf0h, Wf1i, Wf1h = wload_k(wf0h), wload_k(wf1i), wload_k(wf1h)
        Wzp1ob, Wyem = wload_k(wzp1ob, F32), wload_k(wyem)

        # ------------- carries & constants -------------
        def fixed(shape, dt, name):
            return pcar.tile(shape, dt, tag=name, name=name)

        h0bf = fixed([128, 64], F32, "h0bf")
        h0bb = fixed([128, 64], BF, "h0bb")
        h1bf = fixed([128, 64], F32, "h1bf")
        h1bb = fixed([128, 64], BF, "h1bb")
        hff = fixed([128, 128], F32, "hff")   # fwd h: cols 0:64 L0, 64:128 L1
        hfb = fixed([128, 128], BF, "hfb")
        xaA = fixed([97, B], BF, "xaA")
        xaB = fixed([97, B], BF, "xaB")
        xbA = fixed([32, B], BF, "xbA")
        xbB = fixed([32, B], BF, "xbB")
        zbA = fixed([65, B], F32, "zbA")
        zbB = fixed([65, B], F32, "zbB")
        ybwA = fixed([33, B], BF, "ybwA")
        ybwB = fixed([33, B], BF, "ybwB")
        ryb = fixed([65, B], BF, "ryb")
        ones16 = fixed([1, B], BF, "ones16")
        ones16f = fixed([1, B], F32, "ones16f")
        onesN = fixed([1, 512], F32, "onesN")
        halves64 = fixed([64, 1], F32, "halves64")
        ones32f = fixed([32, 1], F32, "ones32f")
        kacc = fixed([64, 1], F32, "kacc")
        nacc = fixed([32, 1], F32, "nacc")

        for tl in (h0bf, h0bb, h1bf, h1bb, hff, hfb,
                   xaA, xaB, xbA, xbB, zbA, zbB, ryb):
            nc.vector.memset(tl[:], 0.0)
        for tl in (xaA, xaB):
            nc.vector.memset(tl[96:97, :], 1.0)
        for tl in (zbA, zbB, ryb):
            nc.vector.memset(tl[64:65, :], 1.0)
        nc.vector.memset(ones16[:], 1.0)
        nc.vector.memset(ones16f[:], 1.0)
        nc.vector.memset(onesN[:], 1.0)
        nc.vector.memset(halves64[:], 0.5)
        nc.vector.memset(ones32f[:], 1.0)
        nc.vector.memset(kacc[:], 0.0)
        nc.vector.memset(nacc[:], 0.0)

        zP = px.tile([128, B], F32, tag="zPpad", name="zPpad")
        nc.vector.memset(zP[:], 0.0)
        nc.sync.dma_start(s_Pe[TT * 128:(TT + 1) * 128, 0:16], zP[:])
        EPC = (TT + 1 + 3) // 4
        for h in range(4):
            lo = h * EPC
            hi = min((h + 1) * EPC, TT + 1)
            if lo >= hi:
                continue
            nc.sync.dma_start(
                s_Pe[lo * 128:hi * 128, 16:32].rearrange("(t p) c -> t p c", p=128),
                epy[lo * 128:hi * 128, :].rearrange("(t f) b -> t f b", f=128))

        # ------------- helpers -------------
        def bank_mms(mm_list):
            """Emit one PSUM accumulation group (one bank). List order is the
            intended execution order; the first MM (start=True, whole-bank
            has_written clear) is forced to execute first and the last
            (stop=True) last via scheduler deps."""
            n = len(mm_list)
            first_inst = None
            insts = []
            for i, (o, l, r) in enumerate(mm_list):
                inst = nc.tensor.matmul(o, l, r, start=(i == 0), stop=(i == n - 1))
                if i > 0:
                    tile.add_dep_helper(inst.ins, first_inst.ins, sync=False,
                                        reason="bank start order")
                if i == n - 1 and n > 2:
                    for other in insts[1:]:
                        tile.add_dep_helper(inst.ins, other.ins, sync=False,
                                            reason="bank stop order")
                if i == 0:
                    first_inst = inst
                insts.append(inst)
            return insts

        def cell(prz, pgin, pghn, hf, hb, b_rz, b_gin, b_ghn):
            """GRU cell nonlinearity; hf/hb are [128,64] APs (fp32/bf16)."""
            if b_rz is not None:
                u = px.tile([128, 128], F32, tag="c_u", name="c_u")
                nc.vector.tensor_tensor(u[:], prz[:], b_rz, OP.add)
                src = u[:]
            else:
                src = prz[:]
            e1 = px.tile([128, 128], F32, tag="c_e1", name="c_e1")
            nc.scalar.activation(e1[:], src, AF.Exp, scale=-1.0)
            l1 = px.tile([128, 128], F32, tag="c_l1", name="c_l1")
            nc.scalar.activation(l1[:], e1[:], AF.Ln, bias=1.0)
            s = px.tile([128, 128], F32, tag="c_s", name="c_s")
            nc.scalar.activation(s[:], l1[:], AF.Exp, scale=-1.0)
            t1 = px.tile([128, 64], F32, tag="c_t1", name="c_t1")
            nc.vector.tensor_tensor(t1[:], pghn[:], b_ghn, OP.add)
            t2 = px.tile([128, 64], F32, tag="c_t2", name="c_t2")
            nc.vector.tensor_tensor(t2[:], s[:, 0:64], t1[:], OP.mult)
            if b_gin is not None:
                t3a = px.tile([128, 64], F32, tag="c_t3a", name="c_t3a")
                nc.vector.tensor_tensor(t3a[:], pgin[:], b_gin, OP.add)
                gin_ap = t3a[:]
            else:
                gin_ap = pgin[:]
            t3 = px.tile([128, 64], F32, tag="c_t3", name="c_t3")
            nc.vector.tensor_tensor(t3[:], t2[:], gin_ap, OP.add)
            e2 = px.tile([128, 64], F32, tag="c_e2", name="c_e2")
            nc.scalar.activation(e2[:], t3[:], AF.Exp, scale=-2.0)
            l2 = px.tile([128, 64], F32, tag="c_l2", name="c_l2")
            nc.scalar.activation(l2[:], e2[:], AF.Ln, bias=1.0)
            sn = px.tile([128, 64], F32, tag="c_sn", name="c_sn")
            nc.scalar.activation(sn[:], l2[:], AF.Exp, scale=-1.0)
            nt = px.tile([128, 64], F32, tag="c_nt", name="c_nt")
            nc.vector.tensor_scalar(nt[:], sn[:], 2.0, -1.0, OP.mult, OP.add)
            d = px.tile([128, 64], F32, tag="c_d", name="c_d")
            nc.vector.tensor_tensor(d[:], hf, nt[:], OP.subtract)
            zd = px.tile([128, 64], F32, tag="c_zd", name="c_zd")
            nc.vector.tensor_tensor(zd[:], s[:, 64:128], d[:], OP.mult)
            nc.vector.tensor_tensor(hf, nt[:], zd[:], OP.add)
            nc.vector.tensor_copy(hb, hf)

        def posterior(tE, zb_cur, zb_nxt, xa_nxt, opost):
            """Posterior/sample for step tE; writes opost cols 0:48, DMAs it."""
            pPe = pp2.tile([128, 32], F32, tag="pPe", name="pPe")
            nc.sync.dma_start(pPe[:], s_Pe[ds(tE * 128, 128)])
            nc.vector.tensor_copy(xa_nxt[0:32, :], pPe[64:96, 16:32])
            q1 = psp.tile([128, B], F32, tag="psmall", name="q1", bufs=3)
            nc.tensor.matmul(q1[:], W["wzp1z"][:], zb_cur[:], start=True, stop=True)
            u1 = px.tile([128, B], F32, tag="p_u1", name="p_u1")
            nc.vector.tensor_tensor(u1[:], q1[:], pPe[:, 0:16], OP.add)
            r1 = px.tile([128, B], F32, tag="p_r1", name="p_r1")
            nc.scalar.activation(r1[:], u1[:], AF.Relu)
            q2 = psp.tile([128, B], F32, tag="psmall", name="q2", bufs=3)
            bank_mms([(q2[:], W["bzp2"][:], ones16f[:]),
                      (q2[:], W["wzp2"][:], r1[:])])
            r2 = px.tile([128, B], F32, tag="p_r2", name="p_r2")
            nc.scalar.activation(r2[:], q2[:], AF.Relu)
            ph = psp.tile([128, B], F32, tag="psmall", name="ph", bufs=3)
            bank_mms([(ph[:], W["bzph"][:], ones16f[:]),
                      (ph[:], W["wzph"][:], r2[:])])
            nc.vector.tensor_copy(opost[:, 0:16], ph[0:64, :])          # pos_m
            eS = px.tile([64, B], F32, tag="p_es", name="p_es")
            nc.scalar.activation(eS[:], ph[64:128, :], AF.Exp)
            nc.scalar.activation(opost[:, 16:32], eS[:], AF.Ln, bias=1.0)  # pos_s
            z1 = px.tile([64, B], F32, tag="p_z1", name="p_z1")
            nc.vector.tensor_tensor(z1[:], pPe[0:64, 16:32], opost[:, 16:32], OP.mult)
            nc.vector.tensor_tensor(opost[:, 32:48], z1[:], opost[:, 0:16], OP.add)
            nc.vector.tensor_copy(zb_nxt[0:64, :], opost[:, 32:48])
            nc.vector.tensor_copy(xa_nxt[32:64, :], opost[0:32, 32:48])
            nc.vector.tensor_copy(xa_nxt[64:96, :], opost[32:64, 32:48])
            nc.sync.dma_start(o_post[ds((tE + 1) * 64, 64)], opost[:])

        def emission(h1b, opost):
            """Emission std for current step t; writes opost[0:32, 48:64]."""
            y1 = psp.tile([64, B], F32, tag="psmall", name="y1", bufs=3)
            bank_mms([(y1[:], W["byem"][:], ones16[:])] +
                     [(y1[:], Wyem[k][:], h1b[:, 16 * k:16 * k + 16])
                      for k in range(HT)])
            ry = px.tile([64, B], F32, tag="e_ry", name="e_ry")
            nc.scalar.activation(ry[:], y1[:], AF.Relu)
            nc.vector.tensor_copy(ryb[0:64, :], ry[:])
            y2 = psp.tile([32, B], F32, tag="psmall", name="y2", bufs=3)
            nc.tensor.matmul(y2[:], W["wyes"][:], ryb[:], start=True, stop=True)
            eY = px.tile([32, B], F32, tag="e_ey", name="e_ey")
            nc.scalar.activation(eY[:], y2[:], AF.Exp)
            nc.scalar.activation(opost[0:32, 48:64], eY[:], AF.Ln, bias=1.0)
            return opost[0:32, 48:64]

        def gate_banks(name):
            prz = psp.tile([128, 128], F32, tag="pgate", name=name + "rz", bufs=4)
            pgin = psp.tile([128, 64], F32, tag="pgate", name=name + "gi", bufs=4)
            pghn = psp.tile([128, 64], F32, tag="pgate", name=name + "gh", bufs=4)
            return prz, pgin, pghn

        def rzs(ps, j):
            return ps[:, 16 * j:16 * j + 16]

        def ns(ps, j):
            return ps[:, 16 * (j - 8):16 * (j - 8) + 16]

        def bwd_step(t, cur_ybw, nxt_ybw):
            nc.sync.dma_start(nxt_ybw[:], ybwd[ds((t + 1) * 33, 33)])
            prz, pgin, pghn = gate_banks("b0")
            bank_mms([(rzs(prz, j), W["wb0a"][:, ts(j, 128)], cur_ybw[:])
                      for j in range(8)] +
                     [(rzs(prz, j), Wb0h[k][:, ts(j, 128)], h0bb[:, ts(k, 16)])
                      for k in range(HT) for j in range(8)])
            bank_mms([(ns(pgin, j), W["wb0a"][:, ts(j, 128)], cur_ybw[:])
                      for j in range(8, 12)])
            bank_mms([(ns(pghn, j), Wb0h[k][:, ts(j, 128)], h0bb[:, ts(k, 16)])
                      for k in range(HT) for j in range(8, 12)])
            cell(prz, pgin, pghn, h0bf[:], h0bb[:], None, None, W["bxb0g"][:])
            prz, pgin, pghn = gate_banks("b1")
            bank_mms([(rzs(prz, j), Wb1h[k][:, ts(j, 128)], h1bb[:, ts(k, 16)])
                      for k in range(HT) for j in range(8)] +
                     [(rzs(prz, j), Wb1i[k][:, ts(j, 128)], h0bb[:, ts(k, 16)])
                      for k in range(HT) for j in range(8)])
            bank_mms([(ns(pgin, j), Wb1i[k][:, ts(j, 128)], h0bb[:, ts(k, 16)])
                      for k in range(HT) for j in range(8, 12)])
            bank_mms([(ns(pghn, j), Wb1h[k][:, ts(j, 128)], h1bb[:, ts(k, 16)])
                      for k in range(HT) for j in range(8, 12)])
            cell(prz, pgin, pghn, h1bf[:], h1bb[:], W["bxb1"][:, 0:128],
                 W["bxb1"][:, 128:192], W["bxb1"][:, 192:256])
            nc.sync.dma_start(s_ob[ds((TT - 1 - t) * 128, 128)], h1bf[:])

        def fwd_step(t, xa_c, xb_c, zb_c, xa_n, xb_n, zb_n):
            opost = px.tile([64, 64], F32, tag="opost", name="opost")
            nc.vector.memset(opost[32:64, 48:64], 0.0)
            prz, pgin, pghn = gate_banks("f0")
            bank_mms([(rzs(prz, j), Wf0h[k][:, ts(j, 128)],
                       hfb[:, 16 * k:16 * k + 16])
                      for k in range(HT) for j in range(8)] +
                     [(rzs(prz, j), W["wf0a"][:, ts(j, 128)], xa_c[:])
                      for j in range(8)] +
                     [(rzs(prz, j), W["wf0b"][:, ts(j, 128)], xb_c[:])
                      for j in range(8)])
            bank_mms([(ns(pgin, j), W["wf0a"][:, ts(j, 128)], xa_c[:])
                      for j in range(8, 12)] +
                     [(ns(pgin, j), W["wf0b"][:, ts(j, 128)], xb_c[:])
                      for j in range(8, 12)])
            bank_mms([(ns(pghn, j), Wf0h[k][:, ts(j, 128)],
                       hfb[:, 16 * k:16 * k + 16])
                      for k in range(HT) for j in range(8, 12)])
            cell(prz, pgin, pghn, hff[:, 0:64], hfb[:, 0:64], None, None,
                 W["bxf0g"][:])
            prz, pgin, pghn = gate_banks("f1")
            bank_mms([(rzs(prz, j), Wf1h[k][:, ts(j, 128)],
                       hfb[:, 64 + 16 * k:64 + 16 * k + 16])
                      for k in range(HT) for j in range(8)] +
                     [(rzs(prz, j), Wf1i[k][:, ts(j, 128)],
                       hfb[:, 16 * k:16 * k + 16])
                      for k in range(HT) for j in range(8)])
            bank_mms([(ns(pgin, j), Wf1i[k][:, ts(j, 128)],
                       hfb[:, 16 * k:16 * k + 16])
                      for k in range(HT) for j in range(8, 12)])
            bank_mms([(ns(pghn, j), Wf1h[k][:, ts(j, 128)],
                       hfb[:, 64 + 16 * k:64 + 16 * k + 16])
                      for k in range(HT) for j in range(8, 12)])
            cell(prz, pgin, pghn, hff[:, 64:128], hfb[:, 64:128],
                 W["bxf1"][:, 0:128], W["bxf1"][:, 128:192],
                 W["bxf1"][:, 192:256])
            nc.sync.dma_start(o_allh[ds(t * 128, 128)], hff[:])
            ys = emission(hfb[:, 64:128], opost)
            nc.vector.tensor_copy(xb_n[:], ys)
            posterior(t + 1, zb_c, zb_n, xa_n, opost)

        # ================= phase 1: backward GRU =================
        nc.sync.dma_start(ybwA[:], ybwd[0:33, :])
        UNB = 2
        with tc.For_i(0, TT, UNB, hint_engines=(ET.PE,)) as t4:
            for u in range(UNB):
                cur, nxt = (ybwA, ybwB) if u % 2 == 0 else (ybwB, ybwA)
                bwd_step(t4 + u, cur, nxt)

        # ================= phase 2: P_ob batched =================
        CP = min(8, TT)
        assert TT % CP == 0
        for c in range(TT // CP):
            obt = pp2.tile([128, CP * 64], F32, tag="obt", name="obt")
            nc.sync.dma_start(
                obt[:].rearrange("p (t c) -> p t c", t=CP),
                s_ob[c * CP * 128:(c + 1) * CP * 128, :].rearrange(
                    "(t p) c -> p t c", p=128))
            psP = psp.tile([128, CP * 16], F32, tag="pgate", name="psP", bufs=4)
            obt3 = obt[:].rearrange("p (t c) -> p t c", t=CP)
            bank_mms([(psP[:], Wzp1ob[k][:], obt3[:, :, 16 * k:16 * k + 16])
                      for k in range(HT)])
            sbP = pp2.tile([128, CP * 16], F32, tag="sbP", name="sbP")
            nc.vector.tensor_copy(sbP[:], psP[:])
            nc.sync.dma_start(
                s_Pe[c * CP * 128:(c + 1) * CP * 128, 0:16].rearrange(
                    "(t p) c -> p t c", p=128),
                sbP[:].rearrange("p (t c) -> p t c", t=CP))

        # ================= phase 3: forward scan =================
        opost0 = px.tile([64, 64], F32, tag="opost", name="opost0")
        nc.vector.memset(opost0[:, 48:64], 0.0)
        posterior(0, zbA, zbA, xaA, opost0)
        UNF = 2
        with tc.For_i(0, TT, UNF, hint_engines=(ET.PE,)) as t2:
            for u in range(UNF):
                if u % 2 == 0:
                    fwd_step(t2 + u, xaA, xbA, zbA, xaB, xbB, zbB)
                else:
                    fwd_step(t2 + u, xaB, xbB, zbB, xaA, xbA, zbA)

        # ================= phase 4: prior + KLD/NLL epilogue =================
        CK = min(16, TT)
        assert TT % CK == 0
        N = CK * B
        for c in range(TT // CK):
            zpf = pp2.tile([64, N], F32, tag="zpf", name="zpf")
            nc.sync.dma_start(
                zpf[:].rearrange("f (t b) -> f t b", t=CK),
                o_post[c * CK * 64:(c + 1) * CK * 64, 32:48].rearrange(
                    "(t f) b -> f t b", f=64))
            zpt = pp2.tile([65, N], F32, tag="zpt", name="zpt")
            nc.vector.tensor_copy(zpt[0:64, :], zpf[:])
            nc.vector.memset(zpt[64:65, :], 1.0)
            pzt = psp.tile([128, N], F32, tag="pgate", name="pzt", bufs=4)
            nc.tensor.matmul(pzt[:], W["wztr"][:], zpt[:], start=True, stop=True)
            rq = pp2.tile([128, N], F32, tag="rq", name="rq")
            nc.scalar.activation(rq[:], pzt[:], AF.Relu)
            phh = psp.tile([128, N], F32, tag="pgate", name="phh", bufs=4)
            bank_mms([(phh[:], W["bzth"][:], onesN[:, 0:N]),
                      (phh[:], W["wzth"][:], rq[:])])
            prm = pp2.tile([64, N], F32, tag="prm", name="prm")
            nc.vector.tensor_copy(prm[:], phh[0:64, :])
            nc.sync.dma_start(
                o_prim[c * CK * 64:(c + 1) * CK * 64, :].rearrange(
                    "(t f) b -> f t b", f=64),
                prm[:].rearrange("f (t b) -> f t b", t=CK))
            pre = pp2.tile([64, N], F32, tag="pre", name="pre")
            nc.scalar.activation(pre[:], phh[64:128, :], AF.Exp)
            prs = pp2.tile([64, N], F32, tag="prs", name="prs")
            nc.scalar.activation(prs[:], pre[:], AF.Ln, bias=1.0)
            nc.sync.dma_start(
                o_pris[c * CK * 64:(c + 1) * CK * 64, :].rearrange(
                    "(t f) b -> f t b", f=64),
                prs[:].rearrange("f (t b) -> f t b", t=CK))
            # KLD terms (pos_m/pos_s read back from o_post)
            pom = pp2.tile([64, N], F32, tag="pom", name="pom")
            nc.sync.dma_start(
                pom[:].rearrange("f (t b) -> f t b", t=CK),
                o_post[(c * CK + 1) * 64:((c + 1) * CK + 1) * 64, 0:16].rearrange(
                    "(t f) b -> f t b", f=64))
            pss = pp2.tile([64, N], F32, tag="pss", name="pss")
            nc.sync.dma_start(
                pss[:].rearrange("f (t b) -> f t b", t=CK),
                o_post[(c * CK + 1) * 64:((c + 1) * CK + 1) * 64, 16:32].rearrange(
                    "(t f) b -> f t b", f=64))
            dm = pp2.tile([64, N], F32, tag="dm", name="dm")
            nc.vector.tensor_tensor(dm[:], pom[:], prm[:], OP.subtract)
            dm2 = pp2.tile([64, N], F32, tag="dm2", name="dm2")
            nc.scalar.activation(dm2[:], dm[:], AF.Square)
            ps2 = pp2.tile([64, N], F32, tag="ps2", name="ps2")
            nc.scalar.activation(ps2[:], pss[:], AF.Square)
            lp = pp2.tile([64, N], F32, tag="lp", name="lp")
            nc.scalar.activation(lp[:], prs[:], AF.Ln)
            ls = pp2.tile([64, N], F32, tag="ls", name="ls")
            nc.scalar.activation(ls[:], pss[:], AF.Ln)
            iv = pp2.tile([64, N], F32, tag="iv", name="iv")
            nc.scalar.activation(iv[:], lp[:], AF.Exp, scale=-2.0)
            num = pp2.tile([64, N], F32, tag="num", name="num")
            nc.vector.tensor_tensor(num[:], ps2[:], dm2[:], OP.add)
            qq = pp2.tile([64, N], F32, tag="qq", name="qq")
            nc.vector.tensor_tensor(qq[:], num[:], iv[:], OP.mult)
            ad = pp2.tile([64, N], F32, tag="ad", name="ad")
            nc.vector.tensor_tensor(ad[:], lp[:], ls[:], OP.subtract)
            ad2 = pp2.tile([64, N], F32, tag="ad2", name="ad2")
            nc.vector.tensor_scalar(ad2[:], ad[:], 2.0, -1.0, OP.mult, OP.add)
            term = pp2.tile([64, N], F32, tag="term", name="term")
            nc.vector.tensor_tensor(term[:], qq[:], ad2[:], OP.add)
            red = pp2.tile([64, 1], F32, tag="red", name="red")
            nc.vector.tensor_reduce(red[:], term[:], mybir.AxisListType.X, OP.add)
            nc.vector.tensor_tensor(kacc[:], kacc[:], red[:], OP.add)
            # NLL terms (ye_s(t) lives at o_post block t+1, cols 48:64)
            ych = pp2.tile([32, N], F32, tag="ych", name="ych")
            nc.sync.dma_start(
                ych[:].rearrange("f (t b) -> f t b", t=CK),
                yT[c * CK * 32:(c + 1) * CK * 32, :].rearrange(
                    "(t f) b -> f t b", f=32))
            ysc = pp2.tile([32, N], F32, tag="ysc", name="ysc")
            nc.sync.dma_start(
                ysc[:].rearrange("f (t b) -> f t b", t=CK),
                o_post[(c * CK + 2) * 64:((c + 1) * CK + 2) * 64, 48:64].rearrange(
                    "(t f) b -> f t b", f=64)[0:32])
            ly = pp2.tile([32, N], F32, tag="ly", name="ly")
            nc.scalar.activation(ly[:], ysc[:], AF.Ln)
            ivy = pp2.tile([32, N], F32, tag="ivy", name="ivy")
            nc.scalar.activation(ivy[:], ly[:], AF.Exp, scale=-2.0)
            y2 = pp2.tile([32, N], F32, tag="y2", name="y2")
            nc.scalar.activation(y2[:], ych[:], AF.Square)
            t5 = pp2.tile([32, N], F32, tag="t5", name="t5")
            nc.vector.tensor_tensor(t5[:], y2[:], ivy[:], OP.mult)
            t6 = pp2.tile([32, N], F32, tag="t6", name="t6")
            nc.vector.tensor_scalar(t6[:], t5[:], 0.5, 0.5 * LOG2PI, OP.mult, OP.add)
            t7 = pp2.tile([32, N], F32, tag="t7", name="t7")
            nc.vector.tensor_tensor(t7[:], t6[:], ly[:], OP.add)
            redn = pp2.tile([32, 1], F32, tag="redn", name="redn")
            nc.vector.tensor_reduce(redn[:], t7[:], mybir.AxisListType.X, OP.add)
            nc.vector.tensor_tensor(nacc[:], nacc[:], redn[:], OP.add)

        pk1 = psp.tile([1, 1], F32, tag="psmall", name="pk1", bufs=3)
        pk2 = psp.tile([1, 1], F32, tag="psmall", name="pk2", bufs=3)
        nc.tensor.matmul(pk1[:], halves64[:], kacc[:], start=True, stop=True)
        nc.tensor.matmul(pk2[:], ones32f[:], nacc[:], start=True, stop=True)
        skn = pp2.tile([1, 2], F32, tag="skn", name="skn")
        nc.vector.tensor_copy(skn[:, 0:1], pk1[:])
        nc.vector.tensor_copy(skn[:, 1:2], pk2[:])
        nc.sync.dma_start(o_kn[:], skn[:])

    nc.compile()
    return nc


# ==================== host-side prep ====================

def _lhsT(w):
    return np.ascontiguousarray(np.asarray(w, np.float32).T)


def _bf(a):
    return np.ascontiguousarray(np.asarray(a, np.float32)).astype(np.float16)


def _expand_bias(b, ncols):
    F = b.shape[0]
    jt = F // 128
    out = np.zeros((128, jt * 16), np.float32)
    for j in range(jt):
        out[:, 16 * j:16 * j + 16] = b[128 * j:128 * j + 128, None]
    assert jt * 16 == ncols
    return out


def prep_weights(inp):
    g = lambda k: np.asarray(inp[k], np.float32)
    w = {}

    def gru_bias(bih, bhh):
        brz = bih[0:2 * HD] + bhh[0:2 * HD]
        row = np.concatenate([brz, bih[2 * HD:]])
        ghn = _expand_bias(bhh[2 * HD:], 64)
        full = np.concatenate([_expand_bias(brz, 128),
                               _expand_bias(bih[2 * HD:], 64), ghn], axis=1)
        return row, ghn, full

    row_b0, ghn_b0, _ = gru_bias(g("bwd_bih0"), g("bwd_bhh0"))
    w["wb0a"] = _bf(np.concatenate([_lhsT(g("bwd_Wih0")), row_b0[None, :]], 0))
    w["wb0h"] = _bf(_lhsT(g("bwd_Whh0")))
    w["bxb0g"] = ghn_b0
    row_b1, _, full_b1 = gru_bias(g("bwd_bih1"), g("bwd_bhh1"))
    w["wb1i"] = _bf(_lhsT(g("bwd_Wih1")))
    w["wb1h"] = _bf(_lhsT(g("bwd_Whh1")))
    w["bxb1"] = full_b1
    row_f0, ghn_f0, _ = gru_bias(g("fwd_bih0"), g("fwd_bhh0"))
    wt = _lhsT(g("fwd_Wih0"))
    w["wf0a"] = _bf(np.concatenate([wt[0:96], row_f0[None, :]], 0))
    w["wf0b"] = _bf(wt[96:128])
    w["wf0h"] = _bf(_lhsT(g("fwd_Whh0")))
    row_f1, _, full_f1 = gru_bias(g("fwd_bih1"), g("fwd_bhh1"))
    w["wf1i"] = _bf(_lhsT(g("fwd_Wih1")))
    w["wf1h"] = _bf(_lhsT(g("fwd_Whh1")))
    w["bxf0g"] = ghn_f0
    w["bxf1"] = full_f1
    z1 = _lhsT(g("zp_W1"))
    w["wzp1ob"] = z1[0:HD]
    w["wzp1z"] = np.ascontiguousarray(
        np.concatenate([z1[HD:HD + ZD], g("zp_b1")[None, :]], 0))
    w["wzp2"] = _lhsT(g("zp_W2"))
    w["bzp2"] = np.ascontiguousarray(g("zp_b2")[None, :])
    w["wzph"] = np.ascontiguousarray(
        np.concatenate([_lhsT(g("zp_mean_W")), _lhsT(g("zp_std_W"))], 1))
    w["bzph"] = np.ascontiguousarray(
        np.concatenate([g("zp_mean_b"), g("zp_std_b")])[None, :])
    w["wyem"] = _bf(_lhsT(g("yem_W")))
    w["byem"] = _bf(g("yem_b")[None, :])
    w["wyes"] = _bf(np.concatenate([_lhsT(g("yem_std_W")), g("yem_std_b")[None, :]], 0))
    w["wztr"] = np.ascontiguousarray(
        np.concatenate([_lhsT(g("ztr_W")), g("ztr_b")[None, :]], 0))
    w["wzth"] = np.ascontiguousarray(
        np.concatenate([_lhsT(g("ztr_mean_W")), _lhsT(g("ztr_std_W"))], 1))
    w["bzth"] = np.ascontiguousarray(
        np.concatenate([g("ztr_mean_b"), g("ztr_std_b")])[None, :])
    return w


def prep_core_inputs(y, eps, wshared, t_steps):
    TT = t_steps
    Bc = y.shape[1]
    m = dict(wshared)
    yr = y[::-1]
    ybwd = np.zeros((TT + 1, 33, Bc), np.float32)
    ybwd[0:TT, 0:32] = yr.transpose(0, 2, 1)
    ybwd[:, 32] = 1.0
    m["ybwd"] = _bf(ybwd.reshape((TT + 1) * 33, Bc))
    epy = np.zeros((TT + 1, 128, Bc), np.float32)
    epy[0:TT, 0:64] = eps.transpose(0, 2, 1)
    epy[1:TT, 64:96] = y[0:TT - 1].transpose(0, 2, 1)
    m["epy"] = np.ascontiguousarray(epy.reshape((TT + 1) * 128, Bc))
    m["yT"] = np.ascontiguousarray(
        y.transpose(0, 2, 1).reshape(TT * 32, Bc).astype(np.float32))
    return m


def unpack_outputs(res, t_steps, Bc):
    TT = t_steps

    def tb(a2, F):  # [(T)*F, Bc] -> [T, Bc, F]
        return a2.reshape(-1, F, Bc).transpose(0, 2, 1)

    po = res["o_post"].reshape(TT + 2, 64, 64)  # [blk, part, col]
    out = {}
    out["pos_m"] = po[1:TT + 1, :, 0:16].transpose(0, 2, 1)
    out["pos_s"] = po[1:TT + 1, :, 16:32].transpose(0, 2, 1)
    out["z"] = po[1:TT + 1, :, 32:48].transpose(0, 2, 1)
    out["ye_s"] = po[2:TT + 2, 0:32, 48:64].transpose(0, 2, 1)
    out["pri_m"] = tb(res["o_prim"], 64)
    out["pri_s"] = tb(res["o_pris"], 64)
    ah = res["o_allh"].reshape(TT, 128, 2, HT, Bc)  # [t, p, l, k, b]
    out["all_h"] = ah.transpose(0, 2, 4, 3, 1).reshape(TT, 2, Bc, HD)
    out["kld"] = float(res["o_kn"][0, 0])
    out["nll"] = float(res["o_kn"][0, 1])
    return out


def _get_nc(t_steps):
    key = ("nc", t_steps)
    if key not in _CACHE:
        _CACHE[key] = build_nc(t_steps)
    return _CACHE[key]


def make_in_maps(inp, t_steps, cores):
    y = np.asarray(inp["y"], np.float32)
    eps = np.asarray(inp["eps"], np.float32)
    Bc = y.shape[1] // cores
    wshared = prep_weights(inp)
    return [prep_core_inputs(y[:, c * Bc:(c + 1) * Bc],
                             eps[:, c * Bc:(c + 1) * Bc], wshared, t_steps)
            for c in range(cores)], Bc


def assemble(results, t_steps, Bc):
    outs = [unpack_outputs(res, t_steps, Bc) for res in results]
    kld = np.float32(sum(o["kld"] for o in outs))
    nll = np.float32(sum(o["nll"] for o in outs))

    def cat(k):
        return np.ascontiguousarray(
            np.concatenate([o[k] for o in outs], axis=-2).astype(np.float32))

    return (kld, nll, cat("pos_m"), cat("pos_s"), cat("pri_m"), cat("pri_s"),
            cat("z"), cat("ye_s"), cat("all_h"))


def get_exec(t_steps=T, cores=NCORES):
    """Build (once) and return (runner, in_names, out_names). runner takes a
    list of global concat arrays (axis 0 = cores) and returns global outputs."""
    key = ("exec", t_steps, cores)
    if key in _CACHE:
        return _CACHE[key]
    import jax
    import jax.numpy as jnp
    import numpy as _np
    from jax.experimental.shard_map import shard_map
    from jax.sharding import Mesh, PartitionSpec
    import concourse.mybir as mybir
    from concourse import bass2jax

    bass2jax.install_neuronx_cc_hook()
    nc = _get_nc(t_steps)
    partition_name = nc.partition_id_tensor.name if nc.partition_id_tensor else None
    in_names, out_names, out_avals = [], [], []
    for alloc in nc.m.functions[0].allocations:
        if not isinstance(alloc, mybir.MemoryLocationSet):
            continue
        name = alloc.memorylocations[0].name
        if alloc.kind == "ExternalInput":
            if name != partition_name:
                in_names.append(name)
        elif alloc.kind == "ExternalOutput":
            out_names.append(name)
            out_avals.append(jax.core.ShapedArray(
                tuple(alloc.tensor_shape), mybir.dt.np(alloc.dtype)))
    all_names = list(in_names) + list(out_names)
    if partition_name is not None:
        all_names.append(partition_name)

    def _body(*args):
        operands = list(args)
        if partition_name is not None:
            operands.append(bass2jax.partition_id_tensor())
        outs = bass2jax._bass_exec_p.bind(
            *operands,
            out_avals=tuple(out_avals),
            in_names=tuple(all_names),
            out_names=tuple(out_names),
            lowering_input_output_aliases=(),
            sim_require_finite=True,
            sim_require_nnan=True,
            nc=nc,
        )
        return tuple(outs)

    devices = jax.devices()[:cores]
    mesh = Mesh(_np.asarray(devices), ("core",))
    n_params = len(in_names)
    n_outs = len(out_names)
    donate = tuple(range(n_params, n_params + n_outs))
    sharded = jax.jit(shard_map(
        _body, mesh=mesh,
        in_specs=(PartitionSpec("core"),) * (n_params + n_outs),
        out_specs=(PartitionSpec("core"),) * n_outs,
        check_rep=False), donate_argnums=donate, keep_unused=True)

    from jax.sharding import NamedSharding
    sh = NamedSharding(mesh, PartitionSpec("core"))
    gshapes = [(av.shape[0] * cores,) + tuple(av.shape[1:]) for av in out_avals]
    gdtypes = [av.dtype for av in out_avals]

    def _mkzeros():
        return tuple(jnp.zeros(s, d) for s, d in zip(gshapes, gdtypes))

    zeros_fn = jax.jit(_mkzeros, out_shardings=tuple(sh for _ in gshapes))
    _CACHE[key] = (sharded, zeros_fn, mesh, in_names, out_names)
    return _CACHE[key]


def device_inputs(in_maps, t_steps=T, cores=NCORES):
    """Concat per-core input dicts along axis 0 and device_put with sharding."""
    import jax
    import numpy as _np
    from jax.sharding import NamedSharding, PartitionSpec
    sharded, zeros_fn, mesh, in_names, out_names = get_exec(t_steps, cores)
    sh = NamedSharding(mesh, PartitionSpec("core"))
    args = []
    for name in in_names:
        g = _np.concatenate([m[name] for m in in_maps], axis=0)
        args.append(jax.device_put(g, sh))
    return args


def run_device(args, t_steps=T, cores=NCORES):
    import jax
    sharded, zeros_fn, mesh, in_names, out_names = get_exec(t_steps, cores)
    zeros = zeros_fn()
    jax.block_until_ready(zeros)
    outs = sharded(*args, *zeros)
    jax.block_until_ready(outs)
    return outs


def run_cores(inp, t_steps=T, cores=NCORES):
    import numpy as _np
    in_maps, Bc = make_in_maps(inp, t_steps, cores)
    args = device_inputs(in_maps, t_steps, cores)
    outs = run_device(args, t_steps, cores)
    sharded, zeros_fn, mesh, in_names, out_names = get_exec(t_steps, cores)
    results = []
    for c in range(cores):
        res = {}
        for name, g in zip(out_names, outs):
            ga = _np.asarray(g)
            rows = ga.shape[0] // cores
            res[name] = ga[c * rows:(c + 1) * rows]
        results.append(res)
    return assemble(results, t_steps, Bc)


def kernel(**inputs):
    return run_cores(inputs, t_steps=T, cores=NCORES)
